# revision 19
# baseline (speedup 1.0000x reference)
import numpy as np
import ml_dtypes
from contextlib import ExitStack

import concourse.mybir as mybir
import concourse.bass as bass
import concourse.tile as tile
from concourse.bass_utils import run_bass_kernel_spmd

# nn_Predictor (moe_routing): L=6 streams, B=16384, D=512, NC=3992, 4 experts.
# Host computes the hard gate (argmax) and routes tokens to their expert; each
# core gets an equal, 128-padded share of every expert's tokens, pre-transposed
# ([feature, token]) and packed in fp8 K-pairs so every matmul runs in fp8
# DoubleRow (2x PE throughput) with no on-device transposes. Weights are
# pre-scaled into e4m3's normal range (x32/x64) and the scales divided back
# out in the psum->sbuf activations, whose f32 biases keep b1/b2 exact. The
# decoder sigmoid uses sigmoid(x) = (1+tanh(x/2))/2 so dec2 multiplies
# centered tanh values; the host adds dec_b2 + 0.5*colsum(dec_W2) at the end.
L, B, D, NCLS, NE = 6, 16384, 512, 3992, 4
NCORES = 8
PAD = 128                   # per-expert per-core column padding
TW = 512                    # column tile width
NCH = (NCLS + 511) // 512   # 8 output column chunks (last = 408)
STREAMS = [(0, 3), (3, 6), (0, 6), (0, 6)]
NK = [3 * D // 128, 3 * D // 128, 6 * D // 128, 6 * D // 128]  # 12,12,24,24
W1S = 32.0                  # fp8 pre-scales
W2S = 64.0
DW1S = 64.0
DW2S = 64.0

F32 = mybir.dt.float32
F8 = mybir.dt.float8e4
F8NP = ml_dtypes.float8_e4m3
DR = mybir.MatmulPerfMode.DoubleRow


def _build(C):
    """C: per-core padded column count per expert (multiples of PAD)."""
    nc = bass.Bass("TRN2")

    xin = {
        e: nc.dram_tensor(f"x{e}", [128, NK[e] * C[e]], F8, kind="ExternalInput")
        for e in range(NE) if C[e]
    }
    w1d = [
        nc.dram_tensor(f"w1_{e}", [128, NK[e] * 512], F8, kind="ExternalInput")
        for e in range(NE)
    ]
    w2d = nc.dram_tensor("w2all", [128, NE * 2048], F8, kind="ExternalInput")
    dw1d = nc.dram_tensor("dw1", [128, 2048], F8, kind="ExternalInput")
    dw2d = nc.dram_tensor("dw2", [128, 4 * NCLS], F8, kind="ExternalInput")
    biasd = nc.dram_tensor("biasp", [128, 36], F32, kind="ExternalInput")
    outD = nc.dram_tensor("out", [sum(C), NCLS], F32, kind="ExternalOutput")

    # column tiles: (expert, global col offset, in-expert offset, width)
    tiles = []
    off = 0
    for e in range(NE):
        for lo in range(0, C[e], TW):
            tiles.append((e, off + lo, lo, min(TW, C[e] - lo)))
        off += C[e]

    with tile.TileContext(nc) as tc, ExitStack() as ctx:
        singles = ctx.enter_context(tc.tile_pool(name="singles", bufs=1))
        xtP = ctx.enter_context(tc.tile_pool(name="xtP", bufs=3))
        hP = ctx.enter_context(tc.tile_pool(name="hP", bufs=4))
        selP = ctx.enter_context(tc.tile_pool(name="selP", bufs=4))
        sigP = ctx.enter_context(tc.tile_pool(name="sigP", bufs=4))
        obP = ctx.enter_context(tc.tile_pool(name="obP", bufs=12))

        hPs = ctx.enter_context(tc.tile_pool(name="hPs", bufs=2, space="PSUM"))
        mPs = ctx.enter_context(tc.tile_pool(name="mPs", bufs=2, space="PSUM"))
        d2Ps = ctx.enter_context(tc.tile_pool(name="d2Ps", bufs=2, space="PSUM"))

        # fp8 pair layouts: lhsT slices are [128, 2, 128], rhs [128, 2, wd]
        w1sb = [
            singles.tile([128, NK[e] // 2, 4, 2, 128], F8, name=f"w1sb{e}")
            for e in range(NE)
        ]
        w2sb = singles.tile([128, NE, 2, 4, 2, 128], F8)
        dw1sb = singles.tile([128, 2, 4, 2, 128], F8)
        dwsb = singles.tile([128, 2, 2, NCLS], F8)
        biassb = singles.tile([128, 36], F32)

        # Load schedule: every startup-critical load rides the sync queue in
        # strict deadline order (in-queue order is priority; concurrent queues
        # round-robin per descriptor and starve small early loads). Weight
        # loads are interleaved between tile emissions; late x tiles go to
        # gpsimd SWDGE, whose slow drain meets their distant deadlines.
        e0_ = tiles[0][0]
        hp0 = NK[e0_] // 4          # half the pairs of the first expert's W1
        nc.sync.dma_start(out=biassb, in_=biasd[:, :])
        nc.sync.dma_start(out=w1sb[e0_][:, :hp0], in_=w1d[e0_][:, :hp0 * 1024])

        rest = []
        seen = {e0_}
        for (e, _, _, _) in tiles:
            if e not in seen:
                seen.add(e)
                rest.append(e)

        def post_tile_loads(ti):
            if ti == 0:
                nc.sync.dma_start(out=w2sb, in_=w2d[:, :])
                nc.sync.dma_start(out=dw1sb, in_=dw1d[:, :])
            elif ti == 1:
                nc.sync.dma_start(
                    out=dwsb,
                    in_=bass.AP(tensor=dw2d, offset=0,
                                ap=[[4 * NCLS, 128], [NCLS, 4], [1, NCLS]]),
                )
                if len(rest) > 0:
                    e = rest[0]
                    nc.sync.dma_start(out=w1sb[e], in_=w1d[e][:, :])
            elif ti == 2:
                for e in rest[1:]:
                    nc.sync.dma_start(out=w1sb[e], in_=w1d[e][:, :])

        def emit_w1w2(ti, e, goff, lo, wd):
            nk = NK[e]
            npair = nk // 2
            xt = xtP.tile([128, npair, 2, wd], F8, name="xt")
            xeng = nc.sync if ti <= 2 else nc.gpsimd
            if ti == 0:
                nh = npair // 2
                nc.sync.dma_start(
                    out=xt[:, :nh, :, :],
                    in_=bass.AP(tensor=xin[e], offset=nk * lo,
                                ap=[[NK[e] * C[e], 128], [2 * wd, nh], [wd, 2], [1, wd]]),
                )
                nc.sync.dma_start(
                    out=w1sb[e][:, hp0:], in_=w1d[e][:, hp0 * 1024:]
                )
                nc.sync.dma_start(
                    out=xt[:, nh:, :, :],
                    in_=bass.AP(tensor=xin[e], offset=nk * lo + nh * 2 * wd,
                                ap=[[NK[e] * C[e], 128], [2 * wd, npair - nh], [wd, 2], [1, wd]]),
                )
            else:
                xeng.dma_start(
                    out=xt,
                    in_=bass.AP(tensor=xin[e], offset=nk * lo,
                                ap=[[NK[e] * C[e], 128], [2 * wd, npair], [wd, 2], [1, wd]]),
                )
            post_tile_loads(ti)

            # W1 (DoubleRow) + relu; h written as x32-scaled fp8 pairs
            hp = [hP.tile([128, 2, wd], F8, name="hp") for _ in range(2)]
            for m in range(4):
                ps = hPs.tile([128, wd], F32, name="hps")
                for p in range(npair):
                    nc.tensor.matmul(
                        ps,
                        w1sb[e][:, p, m],
                        xt[:, p],
                        start=(p == 0),
                        stop=(p == npair - 1),
                        perf_mode=DR,
                    )
                nc.scalar.activation(
                    hp[m // 2][:, m % 2, :], ps, mybir.ActivationFunctionType.Relu,
                    bias=biassb[:, e * 4 + m:e * 4 + m + 1], scale=1.0,
                )

            # W2 (DoubleRow) + b2, scale 1/(32*64) divided out, sel as fp8 pairs
            selp = [selP.tile([128, 2, wd], F8, name="selp") for _ in range(2)]
            for md in range(4):
                ps = mPs.tile([128, wd], F32, name="mps")
                for j in range(2):
                    nc.tensor.matmul(
                        ps, w2sb[:, e, j, md], hp[j],
                        start=(j == 0), stop=(j == 1), perf_mode=DR,
                    )
                nc.scalar.activation(
                    selp[md // 2][:, md % 2, :], ps,
                    mybir.ActivationFunctionType.Identity,
                    bias=biassb[:, 16 + e * 4 + md:16 + e * 4 + md + 1],
                    scale=1.0 / (W1S * W2S),
                )
            return selp

        def emit_dec(selp, goff, wd):
            # dec1 (DoubleRow, x64); tanh((z+db1)/2) in fp8 pairs for dec2
            sigp = [sigP.tile([128, 2, wd], F8, name="sgp") for _ in range(2)]
            for mh in range(4):
                ps = mPs.tile([128, wd], F32, name="mps")
                for j in range(2):
                    nc.tensor.matmul(
                        ps, dw1sb[:, j, mh], selp[j],
                        start=(j == 0), stop=(j == 1), perf_mode=DR,
                    )
                nc.scalar.activation(
                    sigp[mh // 2][:, mh % 2, :], ps,
                    mybir.ActivationFunctionType.Tanh,
                    bias=biassb[:, 32 + mh:32 + mh + 1], scale=0.5 / DW1S,
                )

            # dec2 fp8 DoubleRow; two class chunks per [128, 1024] psum tile so
            # each psum->sbuf copy moves 1024 columns (the copies, not the
            # matmuls, pace this stage — rotate them over all three engines).
            # Class chunks land in a [128, 2048] staging half-row so each
            # token subtile needs two store triggers, not eight.
            for s in range(wd // 128):
                for half in range(2):
                    nws = 2048 if half == 0 else NCLS - 2048
                    ob = obP.tile([128, 2048], F32, name="ob")
                    for q in range(2):
                        base = half * 4 + q * 2
                        w2c = min(1024, NCLS - base * 512)
                        ps2 = d2Ps.tile([128, 1024], F32, name="d2ps")
                        for sub in range(2):
                            n = base + sub
                            nw = min(512, NCLS - n * 512)
                            for j in range(2):
                                nc.tensor.matmul(
                                    ps2[:, sub * 512:sub * 512 + nw],
                                    sigp[j][:, :, s * 128:(s + 1) * 128],
                                    dwsb[:, j, :, n * 512:n * 512 + nw],
                                    start=(j == 0),
                                    stop=(j == 1),
                                    perf_mode=DR,
                                )
                        dst = ob[:, q * 1024:q * 1024 + w2c]
                        if (s * 4 + half * 2 + q) % 2 == 0:
                            nc.vector.tensor_scalar_mul(dst, ps2[:, :w2c], 1.0 / (2 * DW2S))
                        else:
                            nc.scalar.activation(
                                dst, ps2[:, :w2c],
                                mybir.ActivationFunctionType.Copy, scale=1.0 / (2 * DW2S),
                            )
                    nc.scalar.dma_start(
                        out=outD[goff + s * 128:goff + (s + 1) * 128,
                                 half * 2048:half * 2048 + nws],
                        in_=ob[:, :nws],
                    )

        # software pipeline: decoder for tile t-1 is emitted after tile t's
        # expert stage, so the tensor engine never waits on an activation.
        pend = None
        for ti, (e, goff, lo, wd) in enumerate(tiles):
            selp = emit_w1w2(ti, e, goff, lo, wd)
            if pend is not None:
                emit_dec(*pend)
            pend = (selp, goff, wd)
        emit_dec(*pend)

    import bass_rust

    bass_rust.generate_event_semaphores(nc)
    return nc


_NC_CACHE = {}


def _get_nc(C=None):
    if C is None:
        assert _NC_CACHE, "kernel not built yet"
        return next(iter(_NC_CACHE.values()))
    key = tuple(C)
    if key not in _NC_CACHE:
        _NC_CACHE[key] = _build(list(key))
    return _NC_CACHE[key]


def _pair_pack(w, scale):
    # [K, M] -> [128, K/256, M/128, 2, 128] fp8 pairs, flattened to 2D
    K, M = w.shape
    npair, nm = K // 256, M // 128
    return np.ascontiguousarray(
        (w * scale).reshape(npair, 2, 128, nm, 128).transpose(2, 0, 3, 1, 4)
        .reshape(128, K * M // 128).astype(F8NP)
    )


def _route(inputs):
    f32 = np.float32
    x = np.asarray(inputs["fusion_hs"], f32)
    flat = np.transpose(x, (1, 0, 2)).reshape(B, L * D)
    logits = flat.astype(np.float64) @ np.asarray(inputs["gate_W"], f32).astype(
        np.float64
    ) + np.asarray(inputs["gate_b"], f32).astype(np.float64)
    am = np.argmax(logits, axis=1)
    idx = [np.nonzero(am == e)[0] for e in range(NE)]
    percore = [[idx[e][c::NCORES] for c in range(NCORES)] for e in range(NE)]
    C = [
        int(np.ceil(max(len(percore[e][c]) for c in range(NCORES)) / PAD) * PAD)
        if len(idx[e]) else 0
        for e in range(NE)
    ]
    return x, percore, C


def _pack_x(x, idxc, e, Ce):
    l0, l1 = STREAMS[e]
    nl = l1 - l0
    K = nl * D
    nk = K // 128
    n = len(idxc)
    Xe = np.zeros((K, Ce), dtype=F8NP)
    if n:
        Xe[:, :n] = x[l0:l1, idxc, :].transpose(0, 2, 1).reshape(K, n).astype(F8NP)
    blocks = []
    for lo in range(0, Ce, TW):
        wd = min(TW, Ce - lo)
        blocks.append(
            Xe[:, lo:lo + wd].reshape(nk // 2, 2, 128, wd).transpose(2, 0, 1, 3)
            .reshape(128, nk * wd)
        )
    return np.ascontiguousarray(np.concatenate(blocks, axis=1))


def _prep_inputs(inputs):
    f32 = np.float32
    x, percore, C = _route(inputs)

    w13 = np.array(inputs["e3_W1"], f32, copy=True)
    w13[: 3 * D] *= f32(np.asarray(inputs["e3_a"]).reshape(-1)[0])
    w13[3 * D:] *= f32(np.asarray(inputs["e3_b"]).reshape(-1)[0])
    w1s = [np.asarray(inputs["e0_W1"], f32), np.asarray(inputs["e1_W1"], f32),
           np.asarray(inputs["e2_W1"], f32), w13]

    common = {f"w1_{e}": _pair_pack(w1s[e], W1S) for e in range(NE)}
    common["w2all"] = np.concatenate(
        [_pair_pack(np.asarray(inputs[f"e{e}_W2"], f32), W2S) for e in range(NE)],
        axis=1,
    )
    common["dw1"] = _pair_pack(np.asarray(inputs["dec_W1"], f32), DW1S)
    dw2 = np.asarray(inputs["dec_W2"], f32)
    common["dw2"] = np.ascontiguousarray(
        (dw2 * DW2S).reshape(2, 2, 128, NCLS).transpose(2, 0, 1, 3)
        .reshape(128, 4 * NCLS).astype(F8NP)
    )
    b1p = np.concatenate(
        [np.asarray(inputs[f"e{e}_b1"], f32) * f32(W1S) for e in range(NE)]
    ).reshape(16, 128).T
    b2p = np.concatenate(
        [np.asarray(inputs[f"e{e}_b2"], f32) for e in range(NE)]
    ).reshape(16, 128).T
    db1p = (np.asarray(inputs["dec_b1"], f32) * 0.5).reshape(4, 128).T
    common["biasp"] = np.ascontiguousarray(
        np.concatenate([b1p, b2p, db1p], axis=1)
    )

    in_maps = []
    for c in range(NCORES):
        m = dict(common)
        for e in range(NE):
            if C[e]:
                m[f"x{e}"] = _pack_x(x, percore[e][c], e, C[e])
        in_maps.append(m)
    return in_maps, percore, C


def kernel(**inputs):
    in_maps, percore, C = _prep_inputs(inputs)
    nc = _get_nc(C)
    res = run_bass_kernel_spmd(nc, in_maps, core_ids=list(range(NCORES)))
    offs = np.concatenate([[0], np.cumsum(C)])
    out = np.empty((B, NCLS), np.float32)
    for c in range(NCORES):
        r = res.results[c]["out"]
        for e in range(NE):
            ic = percore[e][c]
            if len(ic):
                out[ic] = r[offs[e]:offs[e] + len(ic)]
    # dec2 ran on tanh/2 - 0.5 centered activations; fold the 0.5*colsum(W2)
    # constant and dec_b2 back in here.
    dw2 = np.asarray(inputs["dec_W2"], np.float64)
    out += (np.asarray(inputs["dec_b2"], np.float64).reshape(1, NCLS)
            + 0.5 * dw2.sum(axis=0).reshape(1, NCLS)).astype(np.float32)
    return out


# revision 23
# speedup vs baseline: 1.0946x; 1.0946x over previous
import numpy as np
import ml_dtypes
from contextlib import ExitStack

import concourse.mybir as mybir
import concourse.bass as bass
import concourse.tile as tile
from concourse.bass_utils import run_bass_kernel_spmd

# nn_Predictor (moe_routing): L=6 streams, B=16384, D=512, NC=3992, 4 experts.
# Host computes the hard gate (argmax) and routes tokens to their expert; each
# core gets an equal, 128-padded share of every expert's tokens, pre-transposed
# ([feature, token]) and packed in fp8 K-pairs so every matmul runs in fp8
# DoubleRow (2x PE throughput) with no on-device transposes. Weights are
# pre-scaled into e4m3's normal range (x32/x64) and the scales divided back
# out in the psum->sbuf activations, whose f32 biases keep b1/b2 exact. The
# decoder sigmoid uses sigmoid(x) = (1+tanh(x/2))/2 so dec2 multiplies
# centered tanh values; the host adds dec_b2 + 0.5*colsum(dec_W2) at the end.
L, B, D, NCLS, NE = 6, 16384, 512, 3992, 4
NCORES = 8
PAD = 128                   # per-expert per-core column padding
TW = 512                    # column tile width
NCH = (NCLS + 511) // 512   # 8 output column chunks (last = 408)
STREAMS = [(0, 3), (3, 6), (0, 6), (0, 6)]
NK = [3 * D // 128, 3 * D // 128, 6 * D // 128, 6 * D // 128]  # 12,12,24,24
W1S = 32.0                  # fp8 pre-scales
W2S = 64.0
DW1S = 64.0
DW2S = 64.0

F32 = mybir.dt.float32
F8 = mybir.dt.float8e4
F8NP = ml_dtypes.float8_e4m3
DR = mybir.MatmulPerfMode.DoubleRow


def _build(C):
    """C: per-core padded column count per expert (multiples of PAD)."""
    nc = bass.Bass("TRN2")

    xin = {
        e: nc.dram_tensor(f"x{e}", [128, NK[e] * C[e]], F8, kind="ExternalInput")
        for e in range(NE) if C[e]
    }
    w1d = [
        nc.dram_tensor(f"w1_{e}", [128, NK[e] * 512], F8, kind="ExternalInput")
        for e in range(NE)
    ]
    w2d = nc.dram_tensor("w2all", [128, NE * 2048], F8, kind="ExternalInput")
    dw1d = nc.dram_tensor("dw1", [128, 2048], F8, kind="ExternalInput")
    dw2d = nc.dram_tensor("dw2", [128, 4 * NCLS], F8, kind="ExternalInput")
    biasd = nc.dram_tensor("biasp", [128, 36], F32, kind="ExternalInput")
    outD = nc.dram_tensor("out", [sum(C), NCLS], F32, kind="ExternalOutput")

    # column tiles: (expert, global col offset, in-expert offset, width)
    tiles = []
    off = 0
    for e in range(NE):
        for lo in range(0, C[e], TW):
            tiles.append((e, off + lo, lo, min(TW, C[e] - lo)))
        off += C[e]

    with tile.TileContext(nc) as tc, ExitStack() as ctx:
        singles = ctx.enter_context(tc.tile_pool(name="singles", bufs=1))
        xtP = ctx.enter_context(tc.tile_pool(name="xtP", bufs=3))
        hP = ctx.enter_context(tc.tile_pool(name="hP", bufs=4))
        selP = ctx.enter_context(tc.tile_pool(name="selP", bufs=4))
        sigP = ctx.enter_context(tc.tile_pool(name="sigP", bufs=4))
        obP = ctx.enter_context(tc.tile_pool(name="obP", bufs=12))

        hPs = ctx.enter_context(tc.tile_pool(name="hPs", bufs=2, space="PSUM"))
        mPs = ctx.enter_context(tc.tile_pool(name="mPs", bufs=2, space="PSUM"))
        d2Ps = ctx.enter_context(tc.tile_pool(name="d2Ps", bufs=2, space="PSUM"))

        # fp8 pair layouts: lhsT slices are [128, 2, 128], rhs [128, 2, wd]
        w1sb = [
            singles.tile([128, NK[e] // 2, 4, 2, 128], F8, name=f"w1sb{e}")
            for e in range(NE)
        ]
        w2sb = singles.tile([128, NE, 2, 4, 2, 128], F8)
        dw1sb = singles.tile([128, 2, 4, 2, 128], F8)
        dwsb = singles.tile([128, 2, 2, NCLS], F8)
        biassb = singles.tile([128, 36], F32)

        # Load schedule: every startup-critical load rides the sync queue in
        # strict deadline order (in-queue order is priority; concurrent queues
        # round-robin per descriptor and starve small early loads). Weight
        # loads are interleaved between tile emissions; late x tiles go to
        # gpsimd SWDGE, whose slow drain meets their distant deadlines.
        e0_ = tiles[0][0]
        hp0 = NK[e0_] // 4          # half the pairs of the first expert's W1
        nc.sync.dma_start(out=biassb, in_=biasd[:, :])
        nc.sync.dma_start(out=w1sb[e0_][:, :hp0], in_=w1d[e0_][:, :hp0 * 1024])

        rest = []
        seen = {e0_}
        for (e, _, _, _) in tiles:
            if e not in seen:
                seen.add(e)
                rest.append(e)

        def post_tile_loads(ti):
            if ti == 0:
                nc.sync.dma_start(out=w2sb, in_=w2d[:, :])
                nc.sync.dma_start(out=dw1sb, in_=dw1d[:, :])
            elif ti == 1:
                nc.sync.dma_start(
                    out=dwsb,
                    in_=bass.AP(tensor=dw2d, offset=0,
                                ap=[[4 * NCLS, 128], [NCLS, 4], [1, NCLS]]),
                )
                if len(rest) > 0:
                    e = rest[0]
                    nc.sync.dma_start(out=w1sb[e], in_=w1d[e][:, :])
            elif ti == 2:
                for e in rest[1:]:
                    nc.sync.dma_start(out=w1sb[e], in_=w1d[e][:, :])

        def emit_w1w2(ti, e, goff, lo, wd):
            # generator: yields after each PE atom so the driver can
            # interleave the previous tile's decoder between them
            nk = NK[e]
            npair = nk // 2
            xt = xtP.tile([128, npair, 2, wd], F8, name="xt")
            xeng = nc.sync if ti <= 2 else nc.gpsimd
            if ti == 0:
                nh = npair // 2
                nc.sync.dma_start(
                    out=xt[:, :nh, :, :],
                    in_=bass.AP(tensor=xin[e], offset=nk * lo,
                                ap=[[NK[e] * C[e], 128], [2 * wd, nh], [wd, 2], [1, wd]]),
                )
                nc.sync.dma_start(
                    out=w1sb[e][:, hp0:], in_=w1d[e][:, hp0 * 1024:]
                )
                nc.sync.dma_start(
                    out=xt[:, nh:, :, :],
                    in_=bass.AP(tensor=xin[e], offset=nk * lo + nh * 2 * wd,
                                ap=[[NK[e] * C[e], 128], [2 * wd, npair - nh], [wd, 2], [1, wd]]),
                )
            else:
                xeng.dma_start(
                    out=xt,
                    in_=bass.AP(tensor=xin[e], offset=nk * lo,
                                ap=[[NK[e] * C[e], 128], [2 * wd, npair], [wd, 2], [1, wd]]),
                )
            post_tile_loads(ti)

            # W1 (DoubleRow) + relu; h written as x32-scaled fp8 pairs
            hp = [hP.tile([128, 2, wd], F8, name="hp") for _ in range(2)]
            for m in range(4):
                ps = hPs.tile([128, wd], F32, name="hps")
                for p in range(npair):
                    nc.tensor.matmul(
                        ps,
                        w1sb[e][:, p, m],
                        xt[:, p],
                        start=(p == 0),
                        stop=(p == npair - 1),
                        perf_mode=DR,
                    )
                nc.scalar.activation(
                    hp[m // 2][:, m % 2, :], ps, mybir.ActivationFunctionType.Relu,
                    bias=biassb[:, e * 4 + m:e * 4 + m + 1], scale=1.0,
                )
                yield

            # W2 (DoubleRow) + b2, scale 1/(32*64) divided out, sel as fp8 pairs
            selp = [selP.tile([128, 2, wd], F8, name="selp") for _ in range(2)]
            for md in range(4):
                ps = mPs.tile([128, wd], F32, name="mps")
                for j in range(2):
                    nc.tensor.matmul(
                        ps, w2sb[:, e, j, md], hp[j],
                        start=(j == 0), stop=(j == 1), perf_mode=DR,
                    )
                nc.scalar.activation(
                    selp[md // 2][:, md % 2, :], ps,
                    mybir.ActivationFunctionType.Identity,
                    bias=biassb[:, 16 + e * 4 + md:16 + e * 4 + md + 1],
                    scale=1.0 / (W1S * W2S),
                )
                yield
            return selp

        def emit_dec(selp, goff, wd):
            # dec1 (DoubleRow, x64); tanh((z+db1)/2) in fp8 pairs for dec2
            sigp = [sigP.tile([128, 2, wd], F8, name="sgp") for _ in range(2)]
            for mh in range(4):
                ps = mPs.tile([128, wd], F32, name="mps")
                for j in range(2):
                    nc.tensor.matmul(
                        ps, dw1sb[:, j, mh], selp[j],
                        start=(j == 0), stop=(j == 1), perf_mode=DR,
                    )
                nc.scalar.activation(
                    sigp[mh // 2][:, mh % 2, :], ps,
                    mybir.ActivationFunctionType.Tanh,
                    bias=biassb[:, 32 + mh:32 + mh + 1], scale=0.5 / DW1S,
                )
                yield

            # dec2 fp8 DoubleRow; two class chunks per [128, 1024] psum tile so
            # each psum->sbuf copy moves 1024 columns (the copies, not the
            # matmuls, pace this stage — rotate them over all three engines).
            # Class chunks land in a [128, 2048] staging half-row so each
            # token subtile needs two store triggers, not eight.
            for s in range(wd // 128):
                for half in range(2):
                    nws = 2048 if half == 0 else NCLS - 2048
                    ob = obP.tile([128, 2048], F32, name="ob")
                    for q in range(2):
                        base = half * 4 + q * 2
                        w2c = min(1024, NCLS - base * 512)
                        ps2 = d2Ps.tile([128, 1024], F32, name="d2ps")
                        for sub in range(2):
                            n = base + sub
                            nw = min(512, NCLS - n * 512)
                            for j in range(2):
                                nc.tensor.matmul(
                                    ps2[:, sub * 512:sub * 512 + nw],
                                    sigp[j][:, :, s * 128:(s + 1) * 128],
                                    dwsb[:, j, :, n * 512:n * 512 + nw],
                                    start=(j == 0),
                                    stop=(j == 1),
                                    perf_mode=DR,
                                )
                        dst = ob[:, q * 1024:q * 1024 + w2c]
                        if (s * 4 + half * 2 + q) % 2 == 0:
                            nc.vector.tensor_scalar_mul(dst, ps2[:, :w2c], 1.0 / (2 * DW2S))
                        else:
                            nc.scalar.activation(
                                dst, ps2[:, :w2c],
                                mybir.ActivationFunctionType.Copy, scale=1.0 / (2 * DW2S),
                            )
                        yield
                    nc.scalar.dma_start(
                        out=outD[goff + s * 128:goff + (s + 1) * 128,
                                 half * 2048:half * 2048 + nws],
                        in_=ob[:, :nws],
                    )

        # software pipeline, interleaved at atom granularity: between each
        # expert-stage atom of tile t we emit ~3 decoder atoms of tile t-1,
        # so dec2's copy-paced bubbles are filled with W1/W2 matmuls.
        def drain(gen, k):
            for _ in range(k):
                try:
                    next(gen)
                except StopIteration:
                    return None
            return gen

        pend = None
        for ti, (e, goff, lo, wd) in enumerate(tiles):
            g = emit_w1w2(ti, e, goff, lo, wd)
            selp = None
            while True:
                try:
                    next(g)
                except StopIteration as st:
                    selp = st.value
                    break
                if pend is not None:
                    pend = drain(pend, 3)
            while pend is not None:
                pend = drain(pend, 1)
            pend = emit_dec(selp, goff, wd)
        while pend is not None:
            pend = drain(pend, 1)

    import bass_rust

    bass_rust.generate_event_semaphores(nc)
    return nc


_NC_CACHE = {}


def _get_nc(C=None):
    if C is None:
        assert _NC_CACHE, "kernel not built yet"
        return next(iter(_NC_CACHE.values()))
    key = tuple(C)
    if key not in _NC_CACHE:
        _NC_CACHE[key] = _build(list(key))
    return _NC_CACHE[key]


def _pair_pack(w, scale):
    # [K, M] -> [128, K/256, M/128, 2, 128] fp8 pairs, flattened to 2D
    K, M = w.shape
    npair, nm = K // 256, M // 128
    return np.ascontiguousarray(
        (w * scale).reshape(npair, 2, 128, nm, 128).transpose(2, 0, 3, 1, 4)
        .reshape(128, K * M // 128).astype(F8NP)
    )


def _route(inputs):
    f32 = np.float32
    x = np.asarray(inputs["fusion_hs"], f32)
    flat = np.transpose(x, (1, 0, 2)).reshape(B, L * D)
    logits = flat.astype(np.float64) @ np.asarray(inputs["gate_W"], f32).astype(
        np.float64
    ) + np.asarray(inputs["gate_b"], f32).astype(np.float64)
    am = np.argmax(logits, axis=1)
    idx = [np.nonzero(am == e)[0] for e in range(NE)]
    percore = [[idx[e][c::NCORES] for c in range(NCORES)] for e in range(NE)]
    C = [
        int(np.ceil(max(len(percore[e][c]) for c in range(NCORES)) / PAD) * PAD)
        if len(idx[e]) else 0
        for e in range(NE)
    ]
    return x, percore, C


def _pack_x(x, idxc, e, Ce):
    l0, l1 = STREAMS[e]
    nl = l1 - l0
    K = nl * D
    nk = K // 128
    n = len(idxc)
    Xe = np.zeros((K, Ce), dtype=F8NP)
    if n:
        Xe[:, :n] = x[l0:l1, idxc, :].transpose(0, 2, 1).reshape(K, n).astype(F8NP)
    blocks = []
    for lo in range(0, Ce, TW):
        wd = min(TW, Ce - lo)
        blocks.append(
            Xe[:, lo:lo + wd].reshape(nk // 2, 2, 128, wd).transpose(2, 0, 1, 3)
            .reshape(128, nk * wd)
        )
    return np.ascontiguousarray(np.concatenate(blocks, axis=1))


def _prep_inputs(inputs):
    f32 = np.float32
    x, percore, C = _route(inputs)

    w13 = np.array(inputs["e3_W1"], f32, copy=True)
    w13[: 3 * D] *= f32(np.asarray(inputs["e3_a"]).reshape(-1)[0])
    w13[3 * D:] *= f32(np.asarray(inputs["e3_b"]).reshape(-1)[0])
    w1s = [np.asarray(inputs["e0_W1"], f32), np.asarray(inputs["e1_W1"], f32),
           np.asarray(inputs["e2_W1"], f32), w13]

    common = {f"w1_{e}": _pair_pack(w1s[e], W1S) for e in range(NE)}
    common["w2all"] = np.concatenate(
        [_pair_pack(np.asarray(inputs[f"e{e}_W2"], f32), W2S) for e in range(NE)],
        axis=1,
    )
    common["dw1"] = _pair_pack(np.asarray(inputs["dec_W1"], f32), DW1S)
    dw2 = np.asarray(inputs["dec_W2"], f32)
    common["dw2"] = np.ascontiguousarray(
        (dw2 * DW2S).reshape(2, 2, 128, NCLS).transpose(2, 0, 1, 3)
        .reshape(128, 4 * NCLS).astype(F8NP)
    )
    b1p = np.concatenate(
        [np.asarray(inputs[f"e{e}_b1"], f32) * f32(W1S) for e in range(NE)]
    ).reshape(16, 128).T
    b2p = np.concatenate(
        [np.asarray(inputs[f"e{e}_b2"], f32) for e in range(NE)]
    ).reshape(16, 128).T
    db1p = (np.asarray(inputs["dec_b1"], f32) * 0.5).reshape(4, 128).T
    common["biasp"] = np.ascontiguousarray(
        np.concatenate([b1p, b2p, db1p], axis=1)
    )

    in_maps = []
    for c in range(NCORES):
        m = dict(common)
        for e in range(NE):
            if C[e]:
                m[f"x{e}"] = _pack_x(x, percore[e][c], e, C[e])
        in_maps.append(m)
    return in_maps, percore, C


def kernel(**inputs):
    in_maps, percore, C = _prep_inputs(inputs)
    nc = _get_nc(C)
    res = run_bass_kernel_spmd(nc, in_maps, core_ids=list(range(NCORES)))
    offs = np.concatenate([[0], np.cumsum(C)])
    out = np.empty((B, NCLS), np.float32)
    for c in range(NCORES):
        r = res.results[c]["out"]
        for e in range(NE):
            ic = percore[e][c]
            if len(ic):
                out[ic] = r[offs[e]:offs[e] + len(ic)]
    # dec2 ran on tanh/2 - 0.5 centered activations; fold the 0.5*colsum(W2)
    # constant and dec_b2 back in here.
    dw2 = np.asarray(inputs["dec_W2"], np.float64)
    out += (np.asarray(inputs["dec_b2"], np.float64).reshape(1, NCLS)
            + 0.5 * dw2.sum(axis=0).reshape(1, NCLS)).astype(np.float32)
    return out


# revision 25
# speedup vs baseline: 1.1027x; 1.0074x over previous
import numpy as np
import ml_dtypes
from contextlib import ExitStack

import concourse.mybir as mybir
import concourse.bass as bass
import concourse.tile as tile
from concourse.bass_utils import run_bass_kernel_spmd

# nn_Predictor (moe_routing): L=6 streams, B=16384, D=512, NC=3992, 4 experts.
# Host computes the hard gate (argmax) and routes tokens to their expert; each
# core gets an equal, 128-padded share of every expert's tokens, pre-transposed
# ([feature, token]) and packed in fp8 K-pairs so every matmul runs in fp8
# DoubleRow (2x PE throughput) with no on-device transposes. Weights are
# pre-scaled into e4m3's normal range (x32/x64) and the scales divided back
# out in the psum->sbuf activations, whose f32 biases keep b1/b2 exact. The
# decoder sigmoid uses sigmoid(x) = (1+tanh(x/2))/2 so dec2 multiplies
# centered tanh values; the host adds dec_b2 + 0.5*colsum(dec_W2) at the end.
L, B, D, NCLS, NE = 6, 16384, 512, 3992, 4
NCORES = 8
PAD = 32                    # per-expert per-core column padding
TW = 512                    # column tile width
NCH = (NCLS + 511) // 512   # 8 output column chunks (last = 408)
STREAMS = [(0, 3), (3, 6), (0, 6), (0, 6)]
NK = [3 * D // 128, 3 * D // 128, 6 * D // 128, 6 * D // 128]  # 12,12,24,24
W1S = 32.0                  # fp8 pre-scales
W2S = 64.0
DW1S = 64.0
DW2S = 64.0

F32 = mybir.dt.float32
F8 = mybir.dt.float8e4
F8NP = ml_dtypes.float8_e4m3
DR = mybir.MatmulPerfMode.DoubleRow


def _build(C):
    """C: per-core padded column count per expert (multiples of PAD)."""
    nc = bass.Bass("TRN2")

    xin = {
        e: nc.dram_tensor(f"x{e}", [128, NK[e] * C[e]], F8, kind="ExternalInput")
        for e in range(NE) if C[e]
    }
    w1d = [
        nc.dram_tensor(f"w1_{e}", [128, NK[e] * 512], F8, kind="ExternalInput")
        for e in range(NE)
    ]
    w2d = nc.dram_tensor("w2all", [128, NE * 2048], F8, kind="ExternalInput")
    dw1d = nc.dram_tensor("dw1", [128, 2048], F8, kind="ExternalInput")
    dw2d = nc.dram_tensor("dw2", [128, 4 * NCLS], F8, kind="ExternalInput")
    biasd = nc.dram_tensor("biasp", [128, 36], F32, kind="ExternalInput")
    outD = nc.dram_tensor("out", [sum(C), NCLS], F32, kind="ExternalOutput")

    # column tiles: (expert, global col offset, in-expert offset, width)
    tiles = []
    off = 0
    for e in range(NE):
        for lo in range(0, C[e], TW):
            tiles.append((e, off + lo, lo, min(TW, C[e] - lo)))
        off += C[e]

    with tile.TileContext(nc) as tc, ExitStack() as ctx:
        singles = ctx.enter_context(tc.tile_pool(name="singles", bufs=1))
        xtP = ctx.enter_context(tc.tile_pool(name="xtP", bufs=3))
        hP = ctx.enter_context(tc.tile_pool(name="hP", bufs=4))
        selP = ctx.enter_context(tc.tile_pool(name="selP", bufs=4))
        sigP = ctx.enter_context(tc.tile_pool(name="sigP", bufs=4))
        obP = ctx.enter_context(tc.tile_pool(name="obP", bufs=12))

        hPs = ctx.enter_context(tc.tile_pool(name="hPs", bufs=2, space="PSUM"))
        mPs = ctx.enter_context(tc.tile_pool(name="mPs", bufs=2, space="PSUM"))
        d2Ps = ctx.enter_context(tc.tile_pool(name="d2Ps", bufs=2, space="PSUM"))

        # fp8 pair layouts: lhsT slices are [128, 2, 128], rhs [128, 2, wd]
        w1sb = [
            singles.tile([128, NK[e] // 2, 4, 2, 128], F8, name=f"w1sb{e}")
            for e in range(NE)
        ]
        w2sb = singles.tile([128, NE, 2, 4, 2, 128], F8)
        dw1sb = singles.tile([128, 2, 4, 2, 128], F8)
        dwsb = singles.tile([128, 2, 2, NCLS], F8)
        biassb = singles.tile([128, 36], F32)

        # Load schedule: every startup-critical load rides the sync queue in
        # strict deadline order (in-queue order is priority; concurrent queues
        # round-robin per descriptor and starve small early loads). Weight
        # loads are interleaved between tile emissions; late x tiles go to
        # gpsimd SWDGE, whose slow drain meets their distant deadlines.
        e0_ = tiles[0][0]
        hp0 = NK[e0_] // 4          # half the pairs of the first expert's W1
        nc.sync.dma_start(out=biassb, in_=biasd[:, :])
        nc.sync.dma_start(out=w1sb[e0_][:, :hp0], in_=w1d[e0_][:, :hp0 * 1024])

        rest = []
        seen = {e0_}
        for (e, _, _, _) in tiles:
            if e not in seen:
                seen.add(e)
                rest.append(e)

        def post_tile_loads(ti):
            if ti == 0:
                nc.sync.dma_start(out=w2sb, in_=w2d[:, :])
                nc.sync.dma_start(out=dw1sb, in_=dw1d[:, :])
            elif ti == 1:
                nc.sync.dma_start(
                    out=dwsb,
                    in_=bass.AP(tensor=dw2d, offset=0,
                                ap=[[4 * NCLS, 128], [NCLS, 4], [1, NCLS]]),
                )
                if len(rest) > 0:
                    e = rest[0]
                    nc.sync.dma_start(out=w1sb[e], in_=w1d[e][:, :])
            elif ti == 2:
                for e in rest[1:]:
                    nc.sync.dma_start(out=w1sb[e], in_=w1d[e][:, :])

        def emit_w1w2(ti, e, goff, lo, wd):
            # generator: yields after each PE atom so the driver can
            # interleave the previous tile's decoder between them
            nk = NK[e]
            npair = nk // 2
            xt = xtP.tile([128, npair, 2, wd], F8, name="xt")
            xeng = nc.sync if ti <= 2 else nc.gpsimd
            if ti == 0:
                nh = npair // 2
                nc.sync.dma_start(
                    out=xt[:, :nh, :, :],
                    in_=bass.AP(tensor=xin[e], offset=nk * lo,
                                ap=[[NK[e] * C[e], 128], [2 * wd, nh], [wd, 2], [1, wd]]),
                )
                nc.sync.dma_start(
                    out=w1sb[e][:, hp0:], in_=w1d[e][:, hp0 * 1024:]
                )
                nc.sync.dma_start(
                    out=xt[:, nh:, :, :],
                    in_=bass.AP(tensor=xin[e], offset=nk * lo + nh * 2 * wd,
                                ap=[[NK[e] * C[e], 128], [2 * wd, npair - nh], [wd, 2], [1, wd]]),
                )
            else:
                xeng.dma_start(
                    out=xt,
                    in_=bass.AP(tensor=xin[e], offset=nk * lo,
                                ap=[[NK[e] * C[e], 128], [2 * wd, npair], [wd, 2], [1, wd]]),
                )
            post_tile_loads(ti)

            # W1 (DoubleRow) + relu; h written as x32-scaled fp8 pairs
            hp = [hP.tile([128, 2, wd], F8, name="hp") for _ in range(2)]
            for m in range(4):
                ps = hPs.tile([128, wd], F32, name="hps")
                for p in range(npair):
                    nc.tensor.matmul(
                        ps,
                        w1sb[e][:, p, m],
                        xt[:, p],
                        start=(p == 0),
                        stop=(p == npair - 1),
                        perf_mode=DR,
                    )
                nc.scalar.activation(
                    hp[m // 2][:, m % 2, :], ps, mybir.ActivationFunctionType.Relu,
                    bias=biassb[:, e * 4 + m:e * 4 + m + 1], scale=1.0,
                )
                yield

            # W2 (DoubleRow) + b2, scale 1/(32*64) divided out, sel as fp8 pairs
            selp = [selP.tile([128, 2, wd], F8, name="selp") for _ in range(2)]
            for md in range(4):
                ps = mPs.tile([128, wd], F32, name="mps")
                for j in range(2):
                    nc.tensor.matmul(
                        ps, w2sb[:, e, j, md], hp[j],
                        start=(j == 0), stop=(j == 1), perf_mode=DR,
                    )
                nc.scalar.activation(
                    selp[md // 2][:, md % 2, :], ps,
                    mybir.ActivationFunctionType.Identity,
                    bias=biassb[:, 16 + e * 4 + md:16 + e * 4 + md + 1],
                    scale=1.0 / (W1S * W2S),
                )
                yield
            return selp

        def emit_dec(selp, goff, wd):
            # dec1 (DoubleRow, x64); tanh((z+db1)/2) in fp8 pairs for dec2
            sigp = [sigP.tile([128, 2, wd], F8, name="sgp") for _ in range(2)]
            for mh in range(4):
                ps = mPs.tile([128, wd], F32, name="mps")
                for j in range(2):
                    nc.tensor.matmul(
                        ps, dw1sb[:, j, mh], selp[j],
                        start=(j == 0), stop=(j == 1), perf_mode=DR,
                    )
                nc.scalar.activation(
                    sigp[mh // 2][:, mh % 2, :], ps,
                    mybir.ActivationFunctionType.Tanh,
                    bias=biassb[:, 32 + mh:32 + mh + 1], scale=0.5 / DW1S,
                )
                yield

            # dec2 fp8 DoubleRow; two class chunks per [128, 1024] psum tile so
            # each psum->sbuf copy moves 1024 columns (the copies, not the
            # matmuls, pace this stage — rotate them over all three engines).
            # Class chunks land in a [128, 2048] staging half-row so each
            # token subtile needs two store triggers, not eight.
            for si, s0 in enumerate(range(0, wd, 128)):
                sw = min(128, wd - s0)
                for half in range(2):
                    nws = 2048 if half == 0 else NCLS - 2048
                    ob = obP.tile([128, 2048], F32, name="ob")
                    for q in range(2):
                        base = half * 4 + q * 2
                        w2c = min(1024, NCLS - base * 512)
                        ps2 = d2Ps.tile([128, 1024], F32, name="d2ps")
                        for sub in range(2):
                            n = base + sub
                            nw = min(512, NCLS - n * 512)
                            for j in range(2):
                                nc.tensor.matmul(
                                    ps2[:sw, sub * 512:sub * 512 + nw],
                                    sigp[j][:, :, s0:s0 + sw],
                                    dwsb[:, j, :, n * 512:n * 512 + nw],
                                    start=(j == 0),
                                    stop=(j == 1),
                                    perf_mode=DR,
                                )
                        dst = ob[:sw, q * 1024:q * 1024 + w2c]
                        if (si * 4 + half * 2 + q) % 2 == 0:
                            nc.vector.tensor_scalar_mul(dst, ps2[:sw, :w2c], 1.0 / (2 * DW2S))
                        else:
                            nc.scalar.activation(
                                dst, ps2[:sw, :w2c],
                                mybir.ActivationFunctionType.Copy, scale=1.0 / (2 * DW2S),
                            )
                        yield
                    nc.scalar.dma_start(
                        out=outD[goff + s0:goff + s0 + sw,
                                 half * 2048:half * 2048 + nws],
                        in_=ob[:sw, :nws],
                    )

        # software pipeline, interleaved at atom granularity: between each
        # expert-stage atom of tile t we emit ~3 decoder atoms of tile t-1,
        # so dec2's copy-paced bubbles are filled with W1/W2 matmuls.
        def drain(gen, k):
            for _ in range(k):
                try:
                    next(gen)
                except StopIteration:
                    return None
            return gen

        pend = None
        for ti, (e, goff, lo, wd) in enumerate(tiles):
            g = emit_w1w2(ti, e, goff, lo, wd)
            selp = None
            while True:
                try:
                    next(g)
                except StopIteration as st:
                    selp = st.value
                    break
                if pend is not None:
                    pend = drain(pend, 3)
            while pend is not None:
                pend = drain(pend, 1)
            pend = emit_dec(selp, goff, wd)
        while pend is not None:
            pend = drain(pend, 1)

    import bass_rust

    bass_rust.generate_event_semaphores(nc)
    return nc


_NC_CACHE = {}


def _get_nc(C=None):
    if C is None:
        assert _NC_CACHE, "kernel not built yet"
        return next(iter(_NC_CACHE.values()))
    key = tuple(C)
    if key not in _NC_CACHE:
        _NC_CACHE[key] = _build(list(key))
    return _NC_CACHE[key]


def _pair_pack(w, scale):
    # [K, M] -> [128, K/256, M/128, 2, 128] fp8 pairs, flattened to 2D
    K, M = w.shape
    npair, nm = K // 256, M // 128
    return np.ascontiguousarray(
        (w * scale).reshape(npair, 2, 128, nm, 128).transpose(2, 0, 3, 1, 4)
        .reshape(128, K * M // 128).astype(F8NP)
    )


def _route(inputs):
    f32 = np.float32
    x = np.asarray(inputs["fusion_hs"], f32)
    flat = np.transpose(x, (1, 0, 2)).reshape(B, L * D)
    logits = flat.astype(np.float64) @ np.asarray(inputs["gate_W"], f32).astype(
        np.float64
    ) + np.asarray(inputs["gate_b"], f32).astype(np.float64)
    am = np.argmax(logits, axis=1)
    idx = [np.nonzero(am == e)[0] for e in range(NE)]
    percore = [[idx[e][c::NCORES] for c in range(NCORES)] for e in range(NE)]
    C = [
        int(np.ceil(max(len(percore[e][c]) for c in range(NCORES)) / PAD) * PAD)
        if len(idx[e]) else 0
        for e in range(NE)
    ]
    return x, percore, C


def _pack_x(x, idxc, e, Ce):
    l0, l1 = STREAMS[e]
    nl = l1 - l0
    K = nl * D
    nk = K // 128
    n = len(idxc)
    Xe = np.zeros((K, Ce), dtype=F8NP)
    if n:
        Xe[:, :n] = x[l0:l1, idxc, :].transpose(0, 2, 1).reshape(K, n).astype(F8NP)
    blocks = []
    for lo in range(0, Ce, TW):
        wd = min(TW, Ce - lo)
        blocks.append(
            Xe[:, lo:lo + wd].reshape(nk // 2, 2, 128, wd).transpose(2, 0, 1, 3)
            .reshape(128, nk * wd)
        )
    return np.ascontiguousarray(np.concatenate(blocks, axis=1))


def _prep_inputs(inputs):
    f32 = np.float32
    x, percore, C = _route(inputs)

    w13 = np.array(inputs["e3_W1"], f32, copy=True)
    w13[: 3 * D] *= f32(np.asarray(inputs["e3_a"]).reshape(-1)[0])
    w13[3 * D:] *= f32(np.asarray(inputs["e3_b"]).reshape(-1)[0])
    w1s = [np.asarray(inputs["e0_W1"], f32), np.asarray(inputs["e1_W1"], f32),
           np.asarray(inputs["e2_W1"], f32), w13]

    common = {f"w1_{e}": _pair_pack(w1s[e], W1S) for e in range(NE)}
    common["w2all"] = np.concatenate(
        [_pair_pack(np.asarray(inputs[f"e{e}_W2"], f32), W2S) for e in range(NE)],
        axis=1,
    )
    common["dw1"] = _pair_pack(np.asarray(inputs["dec_W1"], f32), DW1S)
    dw2 = np.asarray(inputs["dec_W2"], f32)
    common["dw2"] = np.ascontiguousarray(
        (dw2 * DW2S).reshape(2, 2, 128, NCLS).transpose(2, 0, 1, 3)
        .reshape(128, 4 * NCLS).astype(F8NP)
    )
    b1p = np.concatenate(
        [np.asarray(inputs[f"e{e}_b1"], f32) * f32(W1S) for e in range(NE)]
    ).reshape(16, 128).T
    b2p = np.concatenate(
        [np.asarray(inputs[f"e{e}_b2"], f32) for e in range(NE)]
    ).reshape(16, 128).T
    db1p = (np.asarray(inputs["dec_b1"], f32) * 0.5).reshape(4, 128).T
    common["biasp"] = np.ascontiguousarray(
        np.concatenate([b1p, b2p, db1p], axis=1)
    )

    in_maps = []
    for c in range(NCORES):
        m = dict(common)
        for e in range(NE):
            if C[e]:
                m[f"x{e}"] = _pack_x(x, percore[e][c], e, C[e])
        in_maps.append(m)
    return in_maps, percore, C


def kernel(**inputs):
    in_maps, percore, C = _prep_inputs(inputs)
    nc = _get_nc(C)
    res = run_bass_kernel_spmd(nc, in_maps, core_ids=list(range(NCORES)))
    offs = np.concatenate([[0], np.cumsum(C)])
    out = np.empty((B, NCLS), np.float32)
    for c in range(NCORES):
        r = res.results[c]["out"]
        for e in range(NE):
            ic = percore[e][c]
            if len(ic):
                out[ic] = r[offs[e]:offs[e] + len(ic)]
    # dec2 ran on tanh/2 - 0.5 centered activations; fold the 0.5*colsum(W2)
    # constant and dec_b2 back in here.
    dw2 = np.asarray(inputs["dec_W2"], np.float64)
    out += (np.asarray(inputs["dec_b2"], np.float64).reshape(1, NCLS)
            + 0.5 * dw2.sum(axis=0).reshape(1, NCLS)).astype(np.float32)
    return out


# revision 26
# speedup vs baseline: 1.1532x; 1.0458x over previous
import numpy as np
import ml_dtypes
from contextlib import ExitStack

import concourse.mybir as mybir
import concourse.bass as bass
import concourse.tile as tile
from concourse.bass_utils import run_bass_kernel_spmd

# nn_Predictor (moe_routing): L=6 streams, B=16384, D=512, NC=3992, 4 experts.
# Host computes the hard gate (argmax) and routes tokens to their expert; each
# core gets an equal, 128-padded share of every expert's tokens, pre-transposed
# ([feature, token]) and packed in fp8 K-pairs so every matmul runs in fp8
# DoubleRow (2x PE throughput) with no on-device transposes. Weights are
# pre-scaled into e4m3's normal range (x32/x64) and the scales divided back
# out in the psum->sbuf activations, whose f32 biases keep b1/b2 exact. The
# decoder sigmoid uses sigmoid(x) = (1+tanh(x/2))/2 so dec2 multiplies
# centered tanh values; the host adds dec_b2 + 0.5*colsum(dec_W2) at the end.
L, B, D, NCLS, NE = 6, 16384, 512, 3992, 4
NCORES = 8
PAD = 32                    # per-expert per-core column padding
TW = 512                    # column tile width
NCH = (NCLS + 511) // 512   # 8 output column chunks (last = 408)
STREAMS = [(0, 3), (3, 6), (0, 6), (0, 6)]
NK = [3 * D // 128, 3 * D // 128, 6 * D // 128, 6 * D // 128]  # 12,12,24,24
W1S = 32.0                  # fp8 pre-scales
W2S = 64.0
DW1S = 64.0
DW2S = 64.0

F32 = mybir.dt.float32
F8 = mybir.dt.float8e4
F8NP = ml_dtypes.float8_e4m3
DR = mybir.MatmulPerfMode.DoubleRow


def _build(C):
    """C: per-core padded column count per expert (multiples of PAD)."""
    nc = bass.Bass("TRN2")

    xin = {
        e: nc.dram_tensor(f"x{e}", [128, NK[e] * C[e]], F8, kind="ExternalInput")
        for e in range(NE) if C[e]
    }
    w1d = [
        nc.dram_tensor(f"w1_{e}", [128, NK[e] * 512], F8, kind="ExternalInput")
        for e in range(NE)
    ]
    w2d = nc.dram_tensor("w2all", [128, NE * 2048], F8, kind="ExternalInput")
    dw1d = nc.dram_tensor("dw1", [128, 2048], F8, kind="ExternalInput")
    dw2d = nc.dram_tensor("dw2", [128, 4 * NCLS], F8, kind="ExternalInput")
    biasd = nc.dram_tensor("biasp", [128, 36], F32, kind="ExternalInput")
    outD = nc.dram_tensor("out", [sum(C), NCLS], F32, kind="ExternalOutput")

    # column tiles: (expert, global col offset, in-expert offset, width)
    tiles = []
    off = 0
    for e in range(NE):
        for lo in range(0, C[e], TW):
            tiles.append((e, off + lo, lo, min(TW, C[e] - lo)))
        off += C[e]

    with tile.TileContext(nc) as tc, ExitStack() as ctx:
        singles = ctx.enter_context(tc.tile_pool(name="singles", bufs=1))
        xtP = ctx.enter_context(tc.tile_pool(name="xtP", bufs=3))
        hP = ctx.enter_context(tc.tile_pool(name="hP", bufs=4))
        selP = ctx.enter_context(tc.tile_pool(name="selP", bufs=4))
        sigP = ctx.enter_context(tc.tile_pool(name="sigP", bufs=4))
        obP = ctx.enter_context(tc.tile_pool(name="obP", bufs=12))

        hPs = ctx.enter_context(tc.tile_pool(name="hPs", bufs=2, space="PSUM"))
        mPs = ctx.enter_context(tc.tile_pool(name="mPs", bufs=2, space="PSUM"))
        d2Ps = ctx.enter_context(tc.tile_pool(name="d2Ps", bufs=2, space="PSUM"))

        # fp8 pair layouts: lhsT slices are [128, 2, 128], rhs [128, 2, wd]
        w1sb = [
            singles.tile([128, NK[e] // 2, 4, 2, 128], F8, name=f"w1sb{e}")
            for e in range(NE)
        ]
        w2sb = singles.tile([128, NE, 2, 4, 2, 128], F8)
        dw1sb = singles.tile([128, 2, 4, 2, 128], F8)
        dwsb = singles.tile([128, 2, 2, NCLS], F8)
        biassb = singles.tile([128, 36], F32)

        # Load schedule: every startup-critical load rides the sync queue in
        # strict deadline order (in-queue order is priority; concurrent queues
        # round-robin per descriptor and starve small early loads). Weight
        # loads are interleaved between tile emissions; late x tiles go to
        # gpsimd SWDGE, whose slow drain meets their distant deadlines.
        e0_ = tiles[0][0]
        hp0 = NK[e0_] // 4          # half the pairs of the first expert's W1
        nc.sync.dma_start(out=biassb, in_=biasd[:, :])
        nc.sync.dma_start(out=w1sb[e0_][:, :hp0], in_=w1d[e0_][:, :hp0 * 1024])

        rest = []
        seen = {e0_}
        for (e, _, _, _) in tiles:
            if e not in seen:
                seen.add(e)
                rest.append(e)

        def post_tile_loads(ti):
            if ti == 0:
                nc.sync.dma_start(out=w2sb, in_=w2d[:, :])
                nc.sync.dma_start(out=dw1sb, in_=dw1d[:, :])
            elif ti == 1:
                nc.sync.dma_start(
                    out=dwsb,
                    in_=bass.AP(tensor=dw2d, offset=0,
                                ap=[[4 * NCLS, 128], [NCLS, 4], [1, NCLS]]),
                )
                if len(rest) > 0:
                    e = rest[0]
                    nc.sync.dma_start(out=w1sb[e], in_=w1d[e][:, :])
            elif ti == 2:
                for e in rest[1:]:
                    nc.sync.dma_start(out=w1sb[e], in_=w1d[e][:, :])

        def emit_w1w2(ti, e, goff, lo, wd):
            # generator: yields after each PE atom so the driver can
            # interleave the previous tile's decoder between them
            nk = NK[e]
            npair = nk // 2
            xt = xtP.tile([128, npair, 2, wd], F8, name="xt")
            xeng = nc.sync if ti <= 2 else nc.gpsimd
            if ti == 0:
                nh = npair // 2
                nc.sync.dma_start(
                    out=xt[:, :nh, :, :],
                    in_=bass.AP(tensor=xin[e], offset=nk * lo,
                                ap=[[NK[e] * C[e], 128], [2 * wd, nh], [wd, 2], [1, wd]]),
                )
                nc.sync.dma_start(
                    out=w1sb[e][:, hp0:], in_=w1d[e][:, hp0 * 1024:]
                )
                nc.sync.dma_start(
                    out=xt[:, nh:, :, :],
                    in_=bass.AP(tensor=xin[e], offset=nk * lo + nh * 2 * wd,
                                ap=[[NK[e] * C[e], 128], [2 * wd, npair - nh], [wd, 2], [1, wd]]),
                )
            else:
                xeng.dma_start(
                    out=xt,
                    in_=bass.AP(tensor=xin[e], offset=nk * lo,
                                ap=[[NK[e] * C[e], 128], [2 * wd, npair], [wd, 2], [1, wd]]),
                )
            post_tile_loads(ti)

            # W1 (DoubleRow) + relu; h written as x32-scaled fp8 pairs
            hp = [hP.tile([128, 2, wd], F8, name="hp") for _ in range(2)]
            for m in range(4):
                ps = hPs.tile([128, wd], F32, name="hps")
                for p in range(npair):
                    nc.tensor.matmul(
                        ps,
                        w1sb[e][:, p, m],
                        xt[:, p],
                        start=(p == 0),
                        stop=(p == npair - 1),
                        perf_mode=DR,
                    )
                nc.scalar.activation(
                    hp[m // 2][:, m % 2, :], ps, mybir.ActivationFunctionType.Relu,
                    bias=biassb[:, e * 4 + m:e * 4 + m + 1], scale=1.0,
                )
                yield

            # W2 (DoubleRow) + b2, scale 1/(32*64) divided out, sel as fp8 pairs
            selp = [selP.tile([128, 2, wd], F8, name="selp") for _ in range(2)]
            for md in range(4):
                ps = mPs.tile([128, wd], F32, name="mps")
                for j in range(2):
                    nc.tensor.matmul(
                        ps, w2sb[:, e, j, md], hp[j],
                        start=(j == 0), stop=(j == 1), perf_mode=DR,
                    )
                nc.scalar.activation(
                    selp[md // 2][:, md % 2, :], ps,
                    mybir.ActivationFunctionType.Identity,
                    bias=biassb[:, 16 + e * 4 + md:16 + e * 4 + md + 1],
                    scale=1.0 / (W1S * W2S),
                )
                yield
            return selp

        def emit_dec(selp, goff, wd):
            # dec1 (DoubleRow, x64); tanh((z+db1)/2) in fp8 pairs for dec2
            sigp = [sigP.tile([128, 2, wd], F8, name="sgp") for _ in range(2)]
            for mh in range(4):
                ps = mPs.tile([128, wd], F32, name="mps")
                for j in range(2):
                    nc.tensor.matmul(
                        ps, dw1sb[:, j, mh], selp[j],
                        start=(j == 0), stop=(j == 1), perf_mode=DR,
                    )
                nc.scalar.activation(
                    sigp[mh // 2][:, mh % 2, :], ps,
                    mybir.ActivationFunctionType.Tanh,
                    bias=biassb[:, 32 + mh:32 + mh + 1], scale=0.5 / DW1S,
                )
                yield

            # dec2 fp8 DoubleRow; two class chunks per [128, 1024] psum tile so
            # each psum->sbuf copy moves 1024 columns (the copies, not the
            # matmuls, pace this stage — rotate them over all three engines).
            # Class chunks land in a [128, 2048] staging half-row so each
            # token subtile needs two store triggers, not eight.
            for si, s0 in enumerate(range(0, wd, 128)):
                sw = min(128, wd - s0)
                for half in range(2):
                    nws = 2048 if half == 0 else NCLS - 2048
                    ob = obP.tile([128, 2048], F32, name="ob")
                    for q in range(2):
                        base = half * 4 + q * 2
                        w2c = min(1024, NCLS - base * 512)
                        ps2 = d2Ps.tile([128, 1024], F32, name="d2ps")
                        for sub in range(2):
                            n = base + sub
                            nw = min(512, NCLS - n * 512)
                            for j in range(2):
                                nc.tensor.matmul(
                                    ps2[:sw, sub * 512:sub * 512 + nw],
                                    sigp[j][:, :, s0:s0 + sw],
                                    dwsb[:, j, :, n * 512:n * 512 + nw],
                                    start=(j == 0),
                                    stop=(j == 1),
                                    perf_mode=DR,
                                )
                        # split the copy across vector AND scalar concurrently:
                        # each half (~0.6us) finishes inside the pair's PE time
                        nc.vector.tensor_scalar_mul(
                            ob[:sw, q * 1024:q * 1024 + 512],
                            ps2[:sw, :512], 1.0 / (2 * DW2S),
                        )
                        if w2c > 512:
                            nc.scalar.activation(
                                ob[:sw, q * 1024 + 512:q * 1024 + w2c],
                                ps2[:sw, 512:w2c],
                                mybir.ActivationFunctionType.Copy, scale=1.0 / (2 * DW2S),
                            )
                        yield
                    nc.scalar.dma_start(
                        out=outD[goff + s0:goff + s0 + sw,
                                 half * 2048:half * 2048 + nws],
                        in_=ob[:sw, :nws],
                    )

        # software pipeline, interleaved at atom granularity: between each
        # expert-stage atom of tile t we emit ~3 decoder atoms of tile t-1,
        # so dec2's copy-paced bubbles are filled with W1/W2 matmuls.
        def drain(gen, k):
            for _ in range(k):
                try:
                    next(gen)
                except StopIteration:
                    return None
            return gen

        pend = None
        for ti, (e, goff, lo, wd) in enumerate(tiles):
            g = emit_w1w2(ti, e, goff, lo, wd)
            selp = None
            while True:
                try:
                    next(g)
                except StopIteration as st:
                    selp = st.value
                    break
                if pend is not None:
                    pend = drain(pend, 3)
            while pend is not None:
                pend = drain(pend, 1)
            pend = emit_dec(selp, goff, wd)
        while pend is not None:
            pend = drain(pend, 1)

    import bass_rust

    bass_rust.generate_event_semaphores(nc)
    return nc


_NC_CACHE = {}


def _get_nc(C=None):
    if C is None:
        assert _NC_CACHE, "kernel not built yet"
        return next(iter(_NC_CACHE.values()))
    key = tuple(C)
    if key not in _NC_CACHE:
        _NC_CACHE[key] = _build(list(key))
    return _NC_CACHE[key]


def _pair_pack(w, scale):
    # [K, M] -> [128, K/256, M/128, 2, 128] fp8 pairs, flattened to 2D
    K, M = w.shape
    npair, nm = K // 256, M // 128
    return np.ascontiguousarray(
        (w * scale).reshape(npair, 2, 128, nm, 128).transpose(2, 0, 3, 1, 4)
        .reshape(128, K * M // 128).astype(F8NP)
    )


def _route(inputs):
    f32 = np.float32
    x = np.asarray(inputs["fusion_hs"], f32)
    flat = np.transpose(x, (1, 0, 2)).reshape(B, L * D)
    logits = flat.astype(np.float64) @ np.asarray(inputs["gate_W"], f32).astype(
        np.float64
    ) + np.asarray(inputs["gate_b"], f32).astype(np.float64)
    am = np.argmax(logits, axis=1)
    idx = [np.nonzero(am == e)[0] for e in range(NE)]
    percore = [[idx[e][c::NCORES] for c in range(NCORES)] for e in range(NE)]
    C = [
        int(np.ceil(max(len(percore[e][c]) for c in range(NCORES)) / PAD) * PAD)
        if len(idx[e]) else 0
        for e in range(NE)
    ]
    return x, percore, C


def _pack_x(x, idxc, e, Ce):
    l0, l1 = STREAMS[e]
    nl = l1 - l0
    K = nl * D
    nk = K // 128
    n = len(idxc)
    Xe = np.zeros((K, Ce), dtype=F8NP)
    if n:
        Xe[:, :n] = x[l0:l1, idxc, :].transpose(0, 2, 1).reshape(K, n).astype(F8NP)
    blocks = []
    for lo in range(0, Ce, TW):
        wd = min(TW, Ce - lo)
        blocks.append(
            Xe[:, lo:lo + wd].reshape(nk // 2, 2, 128, wd).transpose(2, 0, 1, 3)
            .reshape(128, nk * wd)
        )
    return np.ascontiguousarray(np.concatenate(blocks, axis=1))


def _prep_inputs(inputs):
    f32 = np.float32
    x, percore, C = _route(inputs)

    w13 = np.array(inputs["e3_W1"], f32, copy=True)
    w13[: 3 * D] *= f32(np.asarray(inputs["e3_a"]).reshape(-1)[0])
    w13[3 * D:] *= f32(np.asarray(inputs["e3_b"]).reshape(-1)[0])
    w1s = [np.asarray(inputs["e0_W1"], f32), np.asarray(inputs["e1_W1"], f32),
           np.asarray(inputs["e2_W1"], f32), w13]

    common = {f"w1_{e}": _pair_pack(w1s[e], W1S) for e in range(NE)}
    common["w2all"] = np.concatenate(
        [_pair_pack(np.asarray(inputs[f"e{e}_W2"], f32), W2S) for e in range(NE)],
        axis=1,
    )
    common["dw1"] = _pair_pack(np.asarray(inputs["dec_W1"], f32), DW1S)
    dw2 = np.asarray(inputs["dec_W2"], f32)
    common["dw2"] = np.ascontiguousarray(
        (dw2 * DW2S).reshape(2, 2, 128, NCLS).transpose(2, 0, 1, 3)
        .reshape(128, 4 * NCLS).astype(F8NP)
    )
    b1p = np.concatenate(
        [np.asarray(inputs[f"e{e}_b1"], f32) * f32(W1S) for e in range(NE)]
    ).reshape(16, 128).T
    b2p = np.concatenate(
        [np.asarray(inputs[f"e{e}_b2"], f32) for e in range(NE)]
    ).reshape(16, 128).T
    db1p = (np.asarray(inputs["dec_b1"], f32) * 0.5).reshape(4, 128).T
    common["biasp"] = np.ascontiguousarray(
        np.concatenate([b1p, b2p, db1p], axis=1)
    )

    in_maps = []
    for c in range(NCORES):
        m = dict(common)
        for e in range(NE):
            if C[e]:
                m[f"x{e}"] = _pack_x(x, percore[e][c], e, C[e])
        in_maps.append(m)
    return in_maps, percore, C


def kernel(**inputs):
    in_maps, percore, C = _prep_inputs(inputs)
    nc = _get_nc(C)
    res = run_bass_kernel_spmd(nc, in_maps, core_ids=list(range(NCORES)))
    offs = np.concatenate([[0], np.cumsum(C)])
    out = np.empty((B, NCLS), np.float32)
    for c in range(NCORES):
        r = res.results[c]["out"]
        for e in range(NE):
            ic = percore[e][c]
            if len(ic):
                out[ic] = r[offs[e]:offs[e] + len(ic)]
    # dec2 ran on tanh/2 - 0.5 centered activations; fold the 0.5*colsum(W2)
    # constant and dec_b2 back in here.
    dw2 = np.asarray(inputs["dec_W2"], np.float64)
    out += (np.asarray(inputs["dec_b2"], np.float64).reshape(1, NCLS)
            + 0.5 * dw2.sum(axis=0).reshape(1, NCLS)).astype(np.float32)
    return out
